# revision 2
# baseline (speedup 1.0000x reference)
"""GRU kernel for Trainium2: data-parallel over batch across 8 NeuronCores.

Reference: hs = scan(GRU step) over L=512 steps, batch N=64, H=512, E=256.
Each core handles 8 sequences. Per core:
  phase A: gather embeddings (indirect DMA), project all tokens:
           proj = xe @ [Wz_x|Wr_x|Wh_x] + bias  -> DRAM (4096, 1536)
  phase B: 512 sequential steps; per step the recurrent matmuls use the
           transposed hidden state as the PE stationary operand and stream
           the (fp32r) weights as the moving operand; proj[t] is added into
           PSUM via an identity-stationary matmul.
"""
import numpy as np

VOCAB, E, H, L, N = 32000, 256, 512, 512, 64
NCORES = 8
NLOC = N // NCORES  # 8 sequences per core
PSTEPS = 4          # proj ring chunk (steps)
RING = 8            # h ring slots (also output staging)

_cache = {}


def _build(dt_mm_name="float32r"):
    import concourse.bass as bass
    import concourse.mybir as mybir
    import concourse.tile as tile
    from concourse import bacc

    F32 = mybir.dt.float32
    DTM = getattr(mybir.dt, dt_mm_name)
    AF = mybir.ActivationFunctionType
    OP = mybir.AluOpType

    nc = bacc.Bacc("TRN2", target_bir_lowering=False, debug=False)

    x_d = nc.dram_tensor("x_idx", [L * NLOC, 1], mybir.dt.int32, kind="ExternalInput")
    emb_d = nc.dram_tensor("emb", [VOCAB, E], DTM, kind="ExternalInput")
    w_in_d = nc.dram_tensor("w_in", [E, 3 * H], DTM, kind="ExternalInput")
    w_rec_d = nc.dram_tensor("w_rec", [H, 3 * H], DTM, kind="ExternalInput")
    bias_d = nc.dram_tensor("bias", [1, 3 * H], DTM, kind="ExternalInput")
    i8_d = nc.dram_tensor("i8", [8, 8], DTM, kind="ExternalInput")
    i128_d = nc.dram_tensor("i128", [128, 128], DTM, kind="ExternalInput")
    ones_d = nc.dram_tensor("ones", [1, 128], DTM, kind="ExternalInput")
    hs_d = nc.dram_tensor("hs", [L, NLOC, H], F32, kind="ExternalOutput")
    proj_d = nc.dram_tensor("proj", [L * NLOC, 3 * H], DTM)  # internal scratch

    NT = (L * NLOC) // 128  # 32 token tiles

    with tile.TileContext(nc) as tc:
        with tc.tile_pool(name="w", bufs=1) as wp, \
             tc.tile_pool(name="sb", bufs=2) as sb, \
             tc.tile_pool(name="st", bufs=2) as st, \
             tc.tile_pool(name="pr", bufs=2) as pr, \
             tc.tile_pool(name="ps", bufs=2, space="PSUM") as ps:

            # ---- resident weights/constants ----
            w_rec_sb = wp.tile([128, 4, 3 * H], DTM)
            nc.sync.dma_start(w_rec_sb[:], w_rec_d[:].rearrange("(ko ki) c -> ki ko c", ki=128))
            w_in_sb = wp.tile([128, 2, 3 * H], DTM)
            nc.sync.dma_start(w_in_sb[:], w_in_d[:].rearrange("(ko ki) c -> ki ko c", ki=128))
            bias_sb = wp.tile([1, 3 * H], DTM)
            nc.sync.dma_start(bias_sb[:], bias_d[:])
            i8_sb = wp.tile([8, 8], DTM)
            nc.sync.dma_start(i8_sb[:], i8_d[:])
            i128_sb = wp.tile([128, 128], DTM)
            nc.sync.dma_start(i128_sb[:], i128_d[:])
            i8f_sb = wp.tile([8, 8], F32)
            nc.vector.tensor_copy(i8f_sb[:], i8_sb[:])
            ones_sb = wp.tile([1, 128], DTM)
            nc.sync.dma_start(ones_sb[:], ones_d[:])

            # ---- phase A: gather + input projection ----
            for tt in range(NT):
                idx_sb = sb.tile([128, 1], mybir.dt.int32, tag="idx")
                nc.sync.dma_start(idx_sb[:], x_d[128 * tt:128 * (tt + 1), :])
                xe_sb = sb.tile([128, E], DTM, tag="xe")
                nc.gpsimd.indirect_dma_start(
                    out=xe_sb[:], out_offset=None, in_=emb_d[:],
                    in_offset=bass.IndirectOffsetOnAxis(ap=idx_sb[:, :1], axis=0))
                xeT_ps = ps.tile([128, 2, 128], DTM, tag="tp")
                nc.tensor.transpose(xeT_ps[:, 0, :], xe_sb[:, 0:128], i128_sb[:])
                nc.tensor.transpose(xeT_ps[:, 1, :], xe_sb[:, 128:256], i128_sb[:])
                xeT_sb = sb.tile([128, 2, 128], DTM, tag="xeT_sb")
                nc.vector.tensor_copy(xeT_sb[:], xeT_ps[:])
                proj_sb = sb.tile([128, 3 * H], DTM, tag="proj_sb")
                for c in range(3):
                    pp = ps.tile([128, H], F32, tag="ht_ps")
                    cs = slice(H * c, H * (c + 1))
                    nc.tensor.matmul(pp[:], lhsT=ones_sb[:], rhs=bias_sb[:, cs],
                                     start=True, stop=False)
                    nc.tensor.matmul(pp[:], lhsT=xeT_sb[:, 0, :], rhs=w_in_sb[:, 0, cs],
                                     start=False, stop=False)
                    nc.tensor.matmul(pp[:], lhsT=xeT_sb[:, 1, :], rhs=w_in_sb[:, 1, cs],
                                     start=False, stop=True)
                    nc.scalar.activation(proj_sb[:, cs], pp[:], AF.Copy)
                nc.sync.dma_start(proj_d[128 * tt:128 * (tt + 1), :], proj_sb[:])

            # ---- phase B: recurrence ----
            proj_v = proj_d[:].rearrange("(t b) c -> b t c", b=NLOC)  # (8, L, 1536)

            h_ring = st.tile([NLOC, RING, H], F32, tag="h_ring")
            nc.gpsimd.memset(h_ring[:], 0.0)
            z0_sb = st.tile([128, 4, NLOC], F32, tag="z0")
            nc.gpsimd.memset(z0_sb[:], 0.0)
            hT0 = st.tile([128, 4, NLOC], DTM, tag="hT")
            nc.scalar.activation(hT0[:], z0_sb[:], AF.Copy)
            hT_prev = hT0

            for t in range(L):
                if t % PSTEPS == 0:
                    proj_ring = pr.tile([NLOC, PSTEPS, 3 * H], DTM, tag="proj_ring")
                    nc.sync.dma_start(proj_ring[:], proj_v[:, t:t + PSTEPS, :])
                pt = proj_ring[:, t % PSTEPS, :]
                h_prev = h_ring[:, (t - 1) % RING, :] if t > 0 else h_ring[:, RING - 1, :]

                # z,r pre-activations
                zr_ps = ps.tile([NLOC, 2, H], F32, tag="zr_ps")
                for g in range(2):
                    cs = slice(H * g, H * (g + 1))
                    nc.tensor.matmul(zr_ps[:, g, :], lhsT=i8_sb[:], rhs=pt[:, cs],
                                     start=True, stop=False)
                    for k in range(4):
                        nc.tensor.matmul(zr_ps[:, g, :], lhsT=hT_prev[:, k, :],
                                         rhs=w_rec_sb[:, k, cs],
                                         start=False, stop=(k == 3))
                zr_sb = sb.tile([NLOC, 2, H], F32, tag="zr_sb")
                nc.scalar.activation(zr_sb[:], zr_ps[:], AF.Sigmoid)

                # rh = r * h_prev, transposed for the stationary operand
                rh_sb = sb.tile([NLOC, H], DTM, tag="rh_sb")
                nc.vector.tensor_tensor(rh_sb[:], zr_sb[:, 1, :], h_prev, op=OP.mult)
                rhT_ps = ps.tile([128, 4, NLOC], DTM, tag="tp")
                for k in range(4):
                    nc.tensor.transpose(rhT_ps[:, k, :], rh_sb[:, 128 * k:128 * (k + 1)],
                                        i8_sb[:])
                rhT_sb = sb.tile([128, 4, NLOC], DTM, tag="rhT_sb")
                nc.scalar.activation(rhT_sb[:], rhT_ps[:], AF.Copy)

                # h~ pre-activation
                ht_ps = ps.tile([NLOC, H], F32, tag="ht_ps")
                nc.tensor.matmul(ht_ps[:], lhsT=i8_sb[:], rhs=pt[:, 2 * H:3 * H],
                                 start=True, stop=False)
                for k in range(4):
                    nc.tensor.matmul(ht_ps[:], lhsT=rhT_sb[:, k, :],
                                     rhs=w_rec_sb[:, k, 2 * H:3 * H],
                                     start=False, stop=(k == 3))
                htl_sb = sb.tile([NLOC, H], F32, tag="htl_sb")
                nc.scalar.activation(htl_sb[:], ht_ps[:], AF.Tanh)

                # blend: h = h_prev + z*(h~ - h_prev)
                d_sb = sb.tile([NLOC, H], F32, tag="d_sb")
                nc.vector.tensor_tensor(d_sb[:], htl_sb[:], h_prev, op=OP.subtract)
                e_sb = sb.tile([NLOC, H], F32, tag="e_sb")
                nc.vector.tensor_tensor(e_sb[:], zr_sb[:, 0, :], d_sb[:], op=OP.mult)
                h_new = h_ring[:, t % RING, :]
                nc.vector.tensor_tensor(h_new, h_prev, e_sb[:], op=OP.add)

                # transpose h for next step's stationary
                hT_ps = ps.tile([128, 4, NLOC], F32, tag="tp")
                for k in range(4):
                    nc.tensor.transpose(hT_ps[:, k, :], h_new[:, 128 * k:128 * (k + 1)],
                                        i8f_sb[:])
                hT_new = st.tile([128, 4, NLOC], DTM, tag="hT")
                nc.scalar.activation(hT_new[:], hT_ps[:], AF.Copy)
                hT_prev = hT_new

                if t % RING == RING - 1:
                    nc.sync.dma_start(
                        hs_d[t - RING + 1:t + 1, :, :].rearrange("s b h -> b s h"),
                        h_ring[:])
    nc.compile()
    return nc


def _get_nc():
    if "nc" not in _cache:
        _cache["nc"] = _build()
    return _cache["nc"]


def _in_maps(inputs):
    x = np.asarray(inputs["x"])
    emb = np.asarray(inputs["emb"], dtype=np.float32)
    Wz, Wr, Wh = (np.asarray(inputs[k]) for k in ("Wz", "Wr", "Wh"))
    bz, br, bh = (np.asarray(inputs[k]) for k in ("bz", "br", "bh"))
    w_rec = np.concatenate([Wz[:H], Wr[:H], Wh[:H]], axis=1).astype(np.float32)
    w_in = np.concatenate([Wz[H:], Wr[H:], Wh[H:]], axis=1).astype(np.float32)
    bias = np.concatenate([bz, br, bh]).astype(np.float32)[None, :]
    i8 = np.eye(8, dtype=np.float32)
    i128 = np.eye(128, dtype=np.float32)
    ones = np.ones((1, 128), dtype=np.float32)

    in_maps = []
    for c in range(NCORES):
        xc = np.ascontiguousarray(x[:, NLOC * c:NLOC * (c + 1)]).astype(np.int32)
        in_maps.append({
            "x_idx": xc.reshape(L * NLOC, 1),
            "emb": emb, "w_in": w_in, "w_rec": w_rec, "bias": bias,
            "i8": i8, "i128": i128, "ones": ones,
        })
    return in_maps


def kernel(x, emb, Wz, bz, Wr, br, Wh, bh):
    from concourse.bass_utils import run_bass_kernel_spmd

    in_maps = _in_maps(dict(x=x, emb=emb, Wz=Wz, bz=bz, Wr=Wr, br=br,
                            Wh=Wh, bh=bh))
    nc = _get_nc()
    res = run_bass_kernel_spmd(nc, in_maps, core_ids=list(range(NCORES)))
    out = np.empty((L, N, H), dtype=np.float32)
    for c in range(NCORES):
        out[:, NLOC * c:NLOC * (c + 1), :] = res.results[c]["hs"]
    return out

